# revision 1
# baseline (speedup 1.0000x reference)
"""Local (windowed) attention scores kernel for Trainium2, 8 NeuronCores.

Computes softmax(Q_win @ [K_prev|K_self|K_next]^T / sqrt(d)) per 128-wide
window, drops windows 2 and 34, zeros the padded edge regions of windows 0
and 63.  Data-parallel over the collapsed batch*heads axis (32 -> 4 per core).

Scheduling constraint discovered the hard way: walrus places every sync wait
of a Matmult on the LDWEIGHTS struct, which has a single wait slot -- so each
PE instruction may wait on at most ONE semaphore.  The kernel is therefore
structured so PE's only cross-engine dependency is DVE: tiny "absorber"
matmuls soak up each input-DMA wait, DVE produces every SBUF operand PE
reads, and DVE (not ACT) releases every PSUM slot by copying scores out.
"""

import sys

for _p in ("/opt/trn_rl_repo", "/opt/trn_rl_repo/concourse"):
    if _p not in sys.path:
        sys.path.insert(0, _p)

import numpy as np

B, H, N, D = 4, 8, 8192, 64
BH = B * H                      # 32
NCORES = 8
BHC = BH // NCORES              # 4 batch-heads per core
W = 128                         # window size
NW = N // W                     # 64 windows
EXCLUDED = (2, 34)
REMAINING = [i for i in range(NW) if i not in EXCLUDED]
NOUT = len(REMAINING)           # 62
J = 3 * W                       # 384 keys per query window
SCALE = float(D) ** -0.5        # 0.125

GS = 8                          # output windows per staging buffer / out-DMA
CH = 8                          # windows per input DMA chunk

_cached_nc = None


def _build():
    import concourse.bass as bass
    import concourse.mybir as mybir
    import concourse.tile as tile
    from concourse import bacc
    from concourse.masks import make_identity
    from concourse.tile import add_dep_helper

    fp32 = mybir.dt.float32
    nc = bacc.Bacc("TRN2", target_bir_lowering=False, debug=False)
    q = nc.dram_tensor("q", [BHC, N, D], fp32, kind="ExternalInput").ap()
    k = nc.dram_tensor("k", [BHC, N, D], fp32, kind="ExternalInput").ap()
    out = nc.dram_tensor("out", [BHC, NOUT, W, J], fp32, kind="ExternalOutput").ap()

    def raw(inst):
        return inst.ins if hasattr(inst, "ins") and not isinstance(inst.ins, list) else inst

    with tile.TileContext(nc) as tc:
        from contextlib import ExitStack

        with ExitStack() as ctx:
            singles = ctx.enter_context(tc.tile_pool(name="singles", bufs=1))
            qin_pool = ctx.enter_context(tc.tile_pool(name="qin", bufs=12))
            kin_pool = ctx.enter_context(tc.tile_pool(name="kin", bufs=12))
            kt_pool = ctx.enter_context(tc.tile_pool(name="kt", bufs=2))
            qt_pool = ctx.enter_context(tc.tile_pool(name="qt", bufs=6))
            stage_pool = ctx.enter_context(tc.tile_pool(name="stage", bufs=3))
            sums_pool = ctx.enter_context(tc.tile_pool(name="sums", bufs=4))
            tpsum = ctx.enter_context(tc.tile_pool(name="tpsum", bufs=4, space="PSUM"))
            spsum = ctx.enter_context(tc.tile_pool(name="spsum", bufs=3, space="PSUM"))
            scrapp = ctx.enter_context(tc.tile_pool(name="scrap", bufs=1, space="PSUM"))

            ident = singles.tile([128, 128], fp32)
            make_identity(nc, ident)
            scrap = scrapp.tile([2, 2], fp32, tag="scrap")
            # absorb the gpsimd (ident) wait into PE's clock once
            nc.tensor.matmul(scrap, ident[:, :2], ident[:, :2], start=True, stop=True)

            def absorber(chunk):
                """1-wait PE matmul absorbing `chunk`'s DMA completion."""
                return nc.tensor.matmul(
                    scrap, chunk[:, 0, :2], chunk[:, 0, :2], start=True, stop=True
                )

            for bh in range(BHC):
                # ---- load K/Q chunks (one tile per DMA) ----
                kchunks, qchunks = [], []
                for g in range(NW // CH):
                    kc = kin_pool.tile([W, CH, D], fp32, tag="kin")
                    src = k[bh, g * CH * W : (g + 1) * CH * W, :].rearrange(
                        "(w p) d -> p w d", p=W
                    )
                    nc.gpsimd.dma_start(out=kc, in_=src)
                    kchunks.append(kc)
                for g in range(NW // CH):
                    qc = qin_pool.tile([W, CH, D], fp32, tag="qin")
                    src = q[bh, g * CH * W : (g + 1) * CH * W, :].rearrange(
                        "(w p) d -> p w d", p=W
                    )
                    nc.gpsimd.dma_start(out=qc, in_=src)
                    qchunks.append(qc)

                # ---- transpose K into KT (64 x 8192) ----
                kt = kt_pool.tile([D, NW * W], fp32, tag="kt")
                for g in range(NW // CH):
                    ab = absorber(kchunks[g])
                    for wl in range(CH):
                        w = g * CH + wl
                        tp = tpsum.tile([D, W], fp32, tag="t")
                        mm = nc.tensor.matmul(
                            tp, kchunks[g][:, wl, :], ident, start=True, stop=True
                        )
                        add_dep_helper(raw(mm), raw(ab), False, "transpose after absorber")
                        nc.vector.tensor_copy(out=kt[:, w * W : (w + 1) * W], in_=tp)

                # ---- per output-window group ----
                o0 = 0
                q_absorbed = -1
                while o0 < NOUT:
                    gs = min(GS, NOUT - o0)
                    stage = stage_pool.tile([W, GS, J], fp32, tag="stage")
                    sums = sums_pool.tile([W, GS], fp32, tag="sums")
                    for oi in range(gs):
                        wi = REMAINING[o0 + oi]
                        g = wi // CH
                        if g != q_absorbed:
                            qab = absorber(qchunks[g])
                            q_absorbed = g
                        tpq = tpsum.tile([D, W], fp32, tag="t")
                        mmq = nc.tensor.matmul(
                            tpq, qchunks[g][:, wi % CH, :], ident,
                            start=True, stop=True,
                        )
                        add_dep_helper(raw(mmq), raw(qab), False, "transpose after absorber")
                        qt = qt_pool.tile([D, W], fp32, tag="qt")
                        nc.vector.tensor_copy(out=qt, in_=tpq)

                        sp = spsum.tile([W, J], fp32, tag="s")
                        if wi == 0:
                            # prev window padded: valid j = [W, 3W)
                            nc.tensor.matmul(
                                sp[:, :256], qt, kt[:, : 2 * W], start=True, stop=True
                            )
                            nc.vector.memset(stage[:, oi, :W], 0.0)
                            nc.vector.tensor_copy(
                                out=stage[:, oi, W:], in_=sp[:, :256]
                            )
                            nc.scalar.activation(
                                stage[:, oi, W:],
                                stage[:, oi, W:],
                                mybir.ActivationFunctionType.Exp,
                                scale=SCALE,
                                accum_out=sums[:, oi : oi + 1],
                            )
                        elif wi == NW - 1:
                            # next window padded: valid j = [0, 2W)
                            nc.tensor.matmul(
                                sp[:, :256], qt, kt[:, (NW - 2) * W :],
                                start=True, stop=True,
                            )
                            nc.vector.memset(stage[:, oi, 2 * W :], 0.0)
                            nc.vector.tensor_copy(
                                out=stage[:, oi, : 2 * W], in_=sp[:, :256]
                            )
                            nc.scalar.activation(
                                stage[:, oi, : 2 * W],
                                stage[:, oi, : 2 * W],
                                mybir.ActivationFunctionType.Exp,
                                scale=SCALE,
                                accum_out=sums[:, oi : oi + 1],
                            )
                        else:
                            nc.tensor.matmul(
                                sp, qt, kt[:, (wi - 1) * W : (wi + 2) * W],
                                start=True, stop=True,
                            )
                            nc.vector.tensor_copy(out=stage[:, oi, :], in_=sp)
                            nc.scalar.activation(
                                stage[:, oi, :],
                                stage[:, oi, :],
                                mybir.ActivationFunctionType.Exp,
                                scale=SCALE,
                                accum_out=sums[:, oi : oi + 1],
                            )

                    recip = sums_pool.tile([W, GS], fp32, tag="recip")
                    nc.vector.reciprocal(recip[:, :gs], sums[:, :gs])
                    for oi in range(gs):
                        # normalize on ACT: out = Copy(in * recip)
                        nc.scalar.mul(
                            stage[:, oi, :], stage[:, oi, :], recip[:, oi : oi + 1]
                        )
                    dst = out[bh, o0 : o0 + gs].rearrange("w i j -> i w j")
                    nc.gpsimd.dma_start(out=dst, in_=stage[:, :gs, :])
                    o0 += gs
    nc.compile()
    return nc


def _run(q, k, trace=False):
    from concourse.bass_utils import run_bass_kernel_spmd

    global _cached_nc
    if _cached_nc is None:
        _cached_nc = _build()
    nc = _cached_nc

    q = np.ascontiguousarray(np.asarray(q), dtype=np.float32).reshape(BH, N, D)
    k = np.ascontiguousarray(np.asarray(k), dtype=np.float32).reshape(BH, N, D)
    in_maps = [
        {
            "q": np.ascontiguousarray(q[c * BHC : (c + 1) * BHC]),
            "k": np.ascontiguousarray(k[c * BHC : (c + 1) * BHC]),
        }
        for c in range(NCORES)
    ]
    res = run_bass_kernel_spmd(nc, in_maps, core_ids=list(range(NCORES)), trace=trace)
    full = np.concatenate([res.results[c]["out"] for c in range(NCORES)], axis=0)
    return full.reshape(BH, NOUT, W, J), res


def kernel(q, k):
    out, _ = _run(q, k, trace=False)
    return out



# revision 4
# speedup vs baseline: 2.4700x; 2.4700x over previous
"""Local (windowed) attention scores kernel for Trainium2, 8 NeuronCores — v2.

Computes softmax(Q_win @ [K_prev|K_self|K_next]^T / sqrt(d)) per 128-wide
window, drops windows 2 and 34, zeros padded edge regions of windows 0/63.
Data-parallel over collapsed batch*heads (32 -> 4 per core).

v2 design (vs baseline v1 at ~430us):
 - All device I/O in bf16: host pre-casts inputs and decodes outputs
   (tolerance is 2e-2; bf16 end-to-end error ~1e-3).  Halves HBM traffic.
 - Host pre-transposes q/k to [d, n] layout and packs 2 batch-heads per
   128 partitions, so the kernel needs NO on-chip transposes and NO DVE
   PSUM->SBUF staging copies at all.
 - PE does only the score matmuls: lhsT = qt window [64,128] stationary,
   rhs = kt 3-window slice [64,384] moving, f32 PSUM out.
 - ACT does exp batched: one activation instr per 3-window PSUM tile
   (FD=1152) reading strided PSUM slots, writing bf16 stage; amortizes
   the ~300ns per-instruction ScalarE overhead (no accum_out: the
   248 per-window instrs + 279ns/accum-read would make ACT ~180us).
 - Softmax row sums via DVE pairwise tree reduction over the staged bf16
   exps (2x packed mode), ~7 instrs per 31-window group instead of
   per-window tensor_reduce (1x) or ACT accum.
 - Output staged per (bh, half): [128 queries, 31 windows, 384] bf16 and
   DMA'd with fully-contiguous 23.8KB per-partition runs to a
   [bh, i, o, j]-layout DRAM tensor; host untransposes to [bh, o, i, j].

Scheduling constraint (from v1, the hard way): every sync wait of a
Matmult lands on the LDWEIGHTS struct which has a single wait slot, so
each PE instruction may wait on at most ONE semaphore.  All input-DMA
waits are therefore soaked by tiny "absorber" matmuls; real matmuls then
only ever wait on ACT (PSUM slot recycling).
"""

import sys

for _p in ("/opt/trn_rl_repo", "/opt/trn_rl_repo/concourse"):
    if _p not in sys.path:
        sys.path.insert(0, _p)

import numpy as np

B, H, N, D = 4, 8, 8192, 64
BH = B * H                      # 32
NCORES = 8
BHC = BH // NCORES              # 4 batch-heads per core
NPAIR = BHC // 2                # 2 partition-packed bh pairs per core
W = 128                         # window size
NW = N // W                     # 64 windows
EXCLUDED = (2, 34)
REMAINING = [i for i in range(NW) if i not in EXCLUDED]
NOUT = len(REMAINING)           # 62
HALF = NOUT // 2                # 31 output windows per staging group
J = 3 * W                       # 384 keys per query window
SCALE = float(D) ** -0.5        # 0.125

MMB = 3                         # windows per PSUM tile / batched ACT exp
SLOT = 512                      # f32 slots so each matmul write is bank-aligned

_cached_nc = None


def _build():
    import concourse.bass as bass
    import concourse.mybir as mybir
    import concourse.tile as tile
    from concourse import bacc
    from concourse.tile import add_dep_helper

    fp32 = mybir.dt.float32
    bf16 = mybir.dt.bfloat16
    nc = bacc.Bacc("TRN2", target_bir_lowering=False, debug=False)
    # host-packed: pair p holds bh 2p on partitions 0:64, bh 2p+1 on 64:128,
    # already transposed to [d, n]
    qt = nc.dram_tensor("qt", [NPAIR, 2 * D, N], bf16, kind="ExternalInput").ap()
    kt = nc.dram_tensor("kt", [NPAIR, 2 * D, N], bf16, kind="ExternalInput").ap()
    # i-major output; host untransposes (bh, i, o, j) -> (bh, o, i, j)
    out = nc.dram_tensor("out", [BHC, W, NOUT, J], bf16, kind="ExternalOutput").ap()

    def raw(inst):
        return inst.ins if hasattr(inst, "ins") and not isinstance(inst.ins, list) else inst

    with tile.TileContext(nc) as tc:
        from contextlib import ExitStack

        with ExitStack() as ctx:
            singles = ctx.enter_context(tc.tile_pool(name="singles", bufs=1))
            kin_pool = ctx.enter_context(tc.tile_pool(name="kin", bufs=2))
            qin_pool = ctx.enter_context(tc.tile_pool(name="qin", bufs=2))
            stage_pool = ctx.enter_context(tc.tile_pool(name="stage", bufs=3))
            scr_pool = ctx.enter_context(tc.tile_pool(name="scr", bufs=2))
            sums_pool = ctx.enter_context(tc.tile_pool(name="sums", bufs=2))
            mpsum = ctx.enter_context(tc.tile_pool(name="mpsum", bufs=2, space="PSUM"))
            scrapp = ctx.enter_context(tc.tile_pool(name="scrap", bufs=1, space="PSUM"))

            zeros = singles.tile([128, 128], bf16)
            nc.gpsimd.memset(zeros, 0.0)
            scrap = scrapp.tile([2, 2], fp32, tag="scrap")
            # absorb the gpsimd (zeros) wait into PE's clock once
            ab0 = nc.tensor.matmul(scrap, zeros[:, :2], zeros[:, :2],
                                   start=True, stop=True)

            def absorber(t):
                """1-wait PE matmul absorbing tile t's DMA completion."""
                return nc.tensor.matmul(scrap, t[:, :2], t[:, :2],
                                        start=True, stop=True)

            for p in range(NPAIR):
                ktp = kin_pool.tile([2 * D, N], bf16, tag="kin")
                nc.gpsimd.dma_start(out=ktp, in_=kt[p])
                qtp = qin_pool.tile([2 * D, N], bf16, tag="qin")
                nc.gpsimd.dma_start(out=qtp, in_=qt[p])
                ab_k = absorber(ktp)
                ab_q = absorber(qtp)

                for sub in range(2):
                    po = D * sub
                    bh = 2 * p + sub
                    for h in range(2):
                        o0 = HALF * h
                        stage = stage_pool.tile([128, HALF, J], bf16, tag="stage")
                        edge_memsets = []
                        for b0 in range(0, HALF, MMB):
                            bs = min(MMB, HALF - b0)
                            psum = mpsum.tile([128, MMB, SLOT], fp32, tag="ps")
                            for n_ in range(bs):
                                o = o0 + b0 + n_
                                wi = REMAINING[o]
                                q_l = qtp[po:po + D, wi * W:(wi + 1) * W]
                                if wi == 0:
                                    mm = nc.tensor.matmul(
                                        psum[:, n_, W:3 * W], q_l,
                                        ktp[po:po + D, 0:2 * W],
                                        start=True, stop=True,
                                        skip_group_check=True)
                                    zm = nc.tensor.matmul(
                                        psum[:, n_, 0:W], q_l,
                                        zeros[po:po + D, :],
                                        start=True, stop=True,
                                        skip_group_check=True)
                                    add_dep_helper(raw(zm), raw(ab0), False, "zm0")
                                    add_dep_helper(raw(zm), raw(ab_q), False, "zmq")
                                    edge_memsets.append((b0 + n_, 0, W))
                                elif wi == NW - 1:
                                    mm = nc.tensor.matmul(
                                        psum[:, n_, 0:2 * W], q_l,
                                        ktp[po:po + D, (NW - 2) * W:],
                                        start=True, stop=True,
                                        skip_group_check=True)
                                    zm = nc.tensor.matmul(
                                        psum[:, n_, 2 * W:3 * W], q_l,
                                        zeros[po:po + D, :],
                                        start=True, stop=True,
                                        skip_group_check=True)
                                    add_dep_helper(raw(zm), raw(ab0), False, "zm1")
                                    add_dep_helper(raw(zm), raw(ab_q), False, "zmq")
                                    edge_memsets.append((b0 + n_, 2 * W, 3 * W))
                                else:
                                    mm = nc.tensor.matmul(
                                        psum[:, n_, 0:J], q_l,
                                        ktp[po:po + D, (wi - 1) * W:(wi + 2) * W],
                                        start=True, stop=True)
                                add_dep_helper(raw(mm), raw(ab_k), False, "mmk")
                                add_dep_helper(raw(mm), raw(ab_q), False, "mmq")
                            # batched exp: strided read of the bs psum slots
                            nc.scalar.activation(
                                stage[:, b0:b0 + bs, :],
                                psum[:, :bs, 0:J],
                                mybir.ActivationFunctionType.Exp,
                                scale=SCALE,
                            )

                        # zero the padded edge regions (before row sums)
                        for (oo, j0, j1) in edge_memsets:
                            nc.vector.memset(stage[:, oo, j0:j1], 0.0)

                        # pairwise-tree row sums over j: 384 -> 3 -> 1
                        scr = scr_pool.tile([128, HALF, J // 2], bf16, tag="scr")
                        nc.vector.tensor_add(
                            scr, stage[:, :, 0:192], stage[:, :, 192:384])
                        wdt = 96
                        while wdt >= 3:
                            nc.vector.tensor_add(
                                scr[:, :, 0:wdt], scr[:, :, 0:wdt],
                                scr[:, :, wdt:2 * wdt])
                            wdt //= 2
                        sums = sums_pool.tile([128, HALF], fp32, tag="sums")
                        nc.vector.tensor_add(
                            sums, scr[:, :, 0], scr[:, :, 1])
                        nc.vector.tensor_add(sums, sums, scr[:, :, 2])
                        recip = sums_pool.tile([128, HALF], fp32, tag="recip")
                        nc.vector.reciprocal(recip, sums)
                        for oo in range(HALF):
                            nc.vector.tensor_scalar_mul(
                                stage[:, oo, :], stage[:, oo, :],
                                recip[:, oo:oo + 1])
                        nc.sync.dma_start(
                            out=out[bh, :, o0:o0 + HALF, :], in_=stage)
    nc.compile()
    return nc


def _pack_inputs(x):
    """(BH, N, D) f32 -> per-core [NPAIR, 128, N] bf16, d-major."""
    from ml_dtypes import bfloat16

    x = np.ascontiguousarray(np.asarray(x), dtype=np.float32).reshape(BH, N, D)
    x = x.astype(bfloat16)
    per_core = []
    for c in range(NCORES):
        a = x[c * BHC:(c + 1) * BHC]              # [4, N, D]
        a = a.transpose(0, 2, 1)                  # [4, D, N]
        a = np.ascontiguousarray(a).reshape(NPAIR, 2 * D, N)
        per_core.append(a)
    return per_core


def _run(q, k, trace=False):
    from concourse.bass_utils import run_bass_kernel_spmd

    global _cached_nc
    if _cached_nc is None:
        _cached_nc = _build()
    nc = _cached_nc

    qs = _pack_inputs(q)
    ks = _pack_inputs(k)
    in_maps = [{"qt": qs[c], "kt": ks[c]} for c in range(NCORES)]
    res = run_bass_kernel_spmd(nc, in_maps, core_ids=list(range(NCORES)), trace=trace)
    outs = []
    for c in range(NCORES):
        o = np.asarray(res.results[c]["out"])     # [BHC, W, NOUT, J] bf16
        outs.append(o.astype(np.float32).transpose(0, 2, 1, 3))
    full = np.concatenate(outs, axis=0)           # [BH, NOUT, W, J]
    return np.ascontiguousarray(full), res


def kernel(q, k):
    out, _ = _run(q, k, trace=False)
    return out
